# revision 10
# baseline (speedup 1.0000x reference)
"""Causal depthwise conv1d (B=8, C=1024, T=8192, K=4, dil=1) on 8 trn2 cores.

Sharding: batch-parallel — core j handles x[j] (1024, 8192), communication-free.

All HBM I/O rides fp16 (host rounds x, upcasts y; conv error ~1e-3 « the 2e-2
gate), halving traffic vs fp32: 32 MiB/core against the ~400 GB/s/core
achievable HBM rate.

Per-core kernel (Bass/Tile), engine budget per 2048-col chunk (32 chunks):
  PE:  taps 1..3 as f16 matmuls per 512-col psum group (lhsT = diag(w[:,k]),
       rhs = the x tile shifted k in the free dim); 12 back-to-back matmuls
       accumulate into a 4-bank [128, 2048] PSUM region (double-buffered).
       On two probe chunks tap 3 runs on GPSIMD instead (scalar_tensor_tensor)
       to measure Q7 elementwise throughput for a future rebalance.
  ACT: one chunk-wide pass tmp = x*w0 + bias (per-partition scale/bias APs).
  DVE: one chunk-wide merge ot = tmp + psum (f16 out), evicting the region.
  DMA: x loads ride the SP HWDGE ring; stores the ACT ring. The weight
       preload is split per channel-block and alternated across both rings
       so each ring carries ~16.4 MiB. Chunk 0 is merged and stored per
       512-col group so the store ring starts ~12us in, not ~24us.
       Tile misses the "store complete before slot reuse" WAR edge for
       ACT-issued DMAs, so it is added explicitly via add_dep_helper.
"""
import numpy as np

import concourse.bacc as bacc
import concourse.mybir as mybir
from concourse.tile import TileContext
from concourse.tile import add_dep_helper
from concourse import bass_utils

B, C, T, K = 8, 1024, 8192, 4
HALO = K - 1          # causal left pad
P = 128               # SBUF partitions
RBLK = C // P         # 8 channel blocks per core
CHUNK = 2048          # time chunk per inner iteration
IOBUFS = 6            # xt pool bufs
OTBUFS = 4            # ot pool bufs (2-chunk tiles; slot-reuse WAR distance)
NCHUNK = T // CHUNK   # 4
NGRP = CHUNK // 512   # psum groups (banks) per chunk
NPE = K - 1           # taps done on PE (1..3); tap 0 rides the ACT pass
NSMALL = 8            # chunks 1..NSMALL-1 store per-chunk (early stream rampup)
GP_PROBE = ()         # (gpsimd TensorScalarPtr is illegal on trn2 Pool engine)
X_DTYPE = "f16"

_cached = {}


def _build():
    nc = bacc.Bacc("TRN2", target_bir_lowering=False, debug=False)
    f32 = mybir.dt.float32
    f16 = mybir.dt.float16

    x_d = nc.dram_tensor("x", [C, T], f16, kind="ExternalInput")
    wd_d = nc.dram_tensor("wd", [P, RBLK * NPE * P], f16, kind="ExternalInput")
    w0_d = nc.dram_tensor("w0", [P, RBLK], f32, kind="ExternalInput")
    w1_d = nc.dram_tensor("w1", [P, RBLK], f32, kind="ExternalInput")
    b_d = nc.dram_tensor("bv", [P, RBLK], f32, kind="ExternalInput")
    y_d = nc.dram_tensor("y", [C, T], f16, kind="ExternalOutput")

    with TileContext(nc) as tc:
        with (
            tc.tile_pool(name="const", bufs=1) as cpool,
            tc.tile_pool(name="io", bufs=IOBUFS) as pool,
            tc.tile_pool(name="ox", bufs=OTBUFS) as opool,
            tc.tile_pool(name="tmp", bufs=3) as tpool,
            tc.tile_pool(name="tmp2", bufs=2) as t2pool,
            tc.tile_pool(name="psum", bufs=2, space="PSUM") as psum_pool,
        ):
            # weight preload split per channel-block, alternating rings, so
            # block 0's taps land in ~0.3us and neither ring carries the
            # whole 0.75 MiB
            wt = cpool.tile([P, RBLK * NPE * P], f16)
            for r in range(RBLK):
                eng = nc.sync if r % 2 == 0 else nc.scalar
                sl = slice(r * NPE * P, (r + 1) * NPE * P)
                eng.dma_start(out=wt[:, sl], in_=wd_d.ap()[:, sl])
            w0t = cpool.tile([P, RBLK], f32)
            nc.sync.dma_start(out=w0t, in_=w0_d.ap())
            w1t = cpool.tile([P, RBLK], f32)
            nc.scalar.dma_start(out=w1t, in_=w1_d.ap())
            bt = cpool.tile([P, RBLK], f32)
            nc.sync.dma_start(out=bt, in_=b_d.ap())

            # ot-slot store DMAs ride the ACT HWDGE ring (parallel to the SP
            # ring carrying loads). Tile misses the WAR edge "store complete
            # before slot reuse" for ACT-issued DMAs, so add it explicitly:
            # the first write into ot tile m waits on tile (m-OTBUFS)'s last
            # store (ACT-ring DMAs are FIFO, so the last store bounds them all).
            tile_last_store = {}
            ot = None
            for r in range(RBLK):
                rows = slice(r * P, (r + 1) * P)
                for i in range(NCHUNK):
                    n = r * NCHUNK + i
                    xt = pool.tile([P, CHUNK + HALO], f16, tag="xt")
                    split = n in (0, RBLK * NCHUNK - 1)
                    if i == 0:
                        # memset doesn't support f16; zero via uint16 view
                        nc.vector.memset(xt[:, 0:HALO].bitcast(mybir.dt.uint16), 0)
                        if split:
                            # head: first matmul group starts after 128KB
                            for s4 in range(NGRP):
                                a = HALO + s4 * 512
                                nc.sync.dma_start(
                                    out=xt[:, a:a + 512],
                                    in_=x_d.ap()[rows, s4 * 512:(s4 + 1) * 512])
                        else:
                            nc.sync.dma_start(out=xt[:, HALO:],
                                              in_=x_d.ap()[rows, 0:CHUNK])
                    elif split:
                        # tail: per-piece loads so the last groups flow
                        # through compute as they land, shrinking the drain
                        nc.sync.dma_start(
                            out=xt[:, 0:HALO + 512],
                            in_=x_d.ap()[rows,
                                         i * CHUNK - HALO:i * CHUNK + 512])
                        for s4 in range(1, NGRP):
                            a = HALO + s4 * 512
                            nc.sync.dma_start(
                                out=xt[:, a:a + 512],
                                in_=x_d.ap()[rows,
                                             i * CHUNK + s4 * 512:
                                             i * CHUNK + (s4 + 1) * 512])
                    else:
                        nc.sync.dma_start(
                            out=xt,
                            in_=x_d.ap()[rows, i * CHUNK - HALO:(i + 1) * CHUNK])

                    gp_tap3 = n in GP_PROBE
                    ps = psum_pool.tile([P, CHUNK], f32, tag="ps")
                    ktop = K - 1 if gp_tap3 else K
                    for s in range(NGRP):
                        for k in range(1, ktop):
                            nc.tensor.matmul(
                                ps[:, s * 512:(s + 1) * 512],
                                wt[:, (r * NPE + k - 1) * P:(r * NPE + k) * P],
                                xt[:, s * 512 + k:s * 512 + k + 512],
                                start=(k == 1), stop=(k == ktop - 1))
                    tmp = tpool.tile([P, CHUNK], f32, tag="tmp")
                    if split:
                        # head/tail chunks: per-group ACT so each merge only
                        # waits on its own 512-col load piece
                        for s in range(NGRP):
                            sl = slice(s * 512, (s + 1) * 512)
                            nc.scalar.activation(
                                tmp[:, sl], xt[:, sl],
                                mybir.ActivationFunctionType.Identity,
                                bias=bt[:, r:r + 1], scale=w0t[:, r:r + 1])
                    else:
                        nc.scalar.activation(
                            tmp, xt[:, 0:CHUNK],
                            mybir.ActivationFunctionType.Identity,
                            bias=bt[:, r:r + 1], scale=w0t[:, r:r + 1])
                    if gp_tap3:
                        # probe: tap 3 on GPSIMD, fused mul-add over the chunk
                        tmp2 = t2pool.tile([P, CHUNK], f32, tag="tmp2")
                        nc.gpsimd.scalar_tensor_tensor(
                            out=tmp2, in0=xt[:, K - 1:K - 1 + CHUNK],
                            scalar=w1t[:, r:r + 1], in1=tmp,
                            op0=mybir.AluOpType.mult, op1=mybir.AluOpType.add)
                        tmp = tmp2

                    m = n // 2
                    if split:
                        # head/tail: merge+store per 512-col group so the
                        # store ring starts early / the tail drains early
                        if i % 2 == 0:
                            ot = opool.tile([P, 2 * CHUNK], f16, tag="ot")
                        half = (i % 2) * CHUNK
                        for s in range(NGRP):
                            sl = slice(s * 512, (s + 1) * 512)
                            osl = slice(half + s * 512, half + (s + 1) * 512)
                            nc.vector.tensor_add(
                                out=ot[:, osl], in0=tmp[:, sl], in1=ps[:, sl])
                            st = nc.scalar.dma_start(
                                out=y_d.ap()[rows,
                                             i * CHUNK + s * 512:
                                             i * CHUNK + (s + 1) * 512],
                                in_=ot[:, osl])
                        tile_last_store[m] = st
                        continue

                    if i % 2 == 0:
                        ot = opool.tile([P, 2 * CHUNK], f16, tag="ot")
                    half = (i % 2) * CHUNK
                    tt = nc.vector.tensor_add(
                        out=ot[:, half:half + CHUNK], in0=tmp, in1=ps)
                    if i % 2 == 0 and m >= OTBUFS:
                        add_dep_helper(
                            tt.ins, tile_last_store[m - OTBUFS].ins,
                            reason="ot slot reuse waits for store DMA")
                    if n < NSMALL or n == RBLK * NCHUNK - 2:
                        # early chunks store per-chunk to ramp the store
                        # ring; chunk 30 stores alone since 31 is split
                        st = nc.scalar.dma_start(
                            out=y_d.ap()[rows, i * CHUNK:(i + 1) * CHUNK],
                            in_=ot[:, half:half + CHUNK])
                        tile_last_store[m] = st
                    elif i % 2 == 1:
                        base = (i - 1) * CHUNK
                        st = nc.scalar.dma_start(
                            out=y_d.ap()[rows, base:base + 2 * CHUNK],
                            in_=ot)
                        tile_last_store[m] = st
    nc.compile()
    return nc


def _host_weights(w, b):
    # wd[p, (r*NPE+k-1)*P + m] = w[r*P+m, 0, k] if p == m else 0 (lhsT diags,
    # taps 1..K-1); tap 0 is applied by the ACT pass via w0; tap 3 by GPSIMD
    # via w1 on probe chunks.
    wd = np.zeros((P, RBLK * NPE * P), dtype=np.float16)
    m = np.arange(P)
    for r in range(RBLK):
        for k in range(1, K):
            wd[m, (r * NPE + k - 1) * P + m] = w[r * P + m, 0, k].astype(np.float16)
    w0 = np.ascontiguousarray(w[:, 0, 0].reshape(RBLK, P).T).astype(np.float32)
    w1 = np.ascontiguousarray(w[:, 0, K - 1].reshape(RBLK, P).T).astype(np.float32)
    bv = np.ascontiguousarray(b.reshape(RBLK, P).T).astype(np.float32)
    return wd, w0, w1, bv


def kernel(x, w, b):
    x = np.asarray(x, dtype=np.float32)
    w = np.asarray(w, dtype=np.float32)
    b = np.asarray(b, dtype=np.float32)

    if "nc" not in _cached:
        _cached["nc"] = _build()
    nc = _cached["nc"]

    wd, w0, w1, bv = _host_weights(w, b)
    x16 = x.astype(np.float16)
    in_maps = [
        {"x": np.ascontiguousarray(x16[j]), "wd": wd, "w0": w0, "w1": w1,
         "bv": bv}
        for j in range(B)
    ]
    res = bass_utils.run_bass_kernel_spmd(nc, in_maps, core_ids=list(range(B)))
    return np.stack([r["y"] for r in res.results], axis=0).astype(np.float32)


# revision 12
# speedup vs baseline: 1.0548x; 1.0548x over previous
"""Causal depthwise conv1d (B=8, C=1024, T=8192, K=4, dil=1) on 8 trn2 cores.

Sharding: batch-parallel — core j handles x[j] (1024, 8192), communication-free.

All HBM I/O rides fp16 (host rounds x, upcasts y; conv error ~1e-3 « the 2e-2
gate), halving traffic vs fp32: 32 MiB/core against the ~400 GB/s/core
achievable HBM rate (~83 us/ring for ~16.4 MiB on each HWDGE ring).

Per-core kernel (Bass/Tile), per 2048-col chunk (32 chunks):
  PE:  taps 1..3 as f16 matmuls per 512-col psum slice (lhsT = diag(w[:,k]),
       rhs = the x tile shifted k in the free dim), accumulating into
       [128, 1024] PSUM granules (4 live = all 8 banks; granule-level deps
       let each merge start after 6 matmuls instead of 12).
  ACT: one chunk-wide pass tmp = x*w0 + bias (per-partition scale/bias APs;
       N=2048 amortizes ACT's ~352-cycle fixed overhead).
  DVE: two per-granule merges ot = tmp + psum (f16 out), evicting PSUM.
  DMA: HWDGE dma_start costs ~600ns of dispatch on the issuing engine, so
       the head of the program keeps dispatch count minimal: all weights
       ride ONE packed f16 tensor split into two loads (scalars+block0
       first), and x loads are whole chunks. Loads ride the SP ring;
       stores ride the ACT ring (per-chunk early to ramp the store ring,
       per-2-chunks in steady state, per-granule at the head/tail).
       Tile misses the "store complete before slot reuse" WAR edge for
       ACT-issued DMAs, so it is added explicitly via add_dep_helper.
"""
import numpy as np

import concourse.bacc as bacc
import concourse.mybir as mybir
from concourse.tile import TileContext
from concourse.tile import add_dep_helper
from concourse import bass_utils

B, C, T, K = 8, 1024, 8192, 4
HALO = K - 1          # causal left pad
P = 128               # SBUF partitions
RBLK = C // P         # 8 channel blocks per core
CHUNK = 2048          # time chunk per inner iteration
GRAN = 1024           # psum granule width (2 banks)
IOBUFS = 6            # xt pool bufs
OTBUFS = 4            # ot pool bufs (2-chunk tiles; slot-reuse WAR distance)
NCHUNK = T // CHUNK   # 4
NTOT = RBLK * NCHUNK  # 32
NPE = K - 1           # taps done on PE (1..3); tap 0 rides the ACT pass
NSMALL = 8            # chunks 1..NSMALL-1 store per-chunk (store-ring rampup)
WCOLS = RBLK * NPE * P       # packed diag-tap weights (f16)
X_DTYPE = "f16"

_cached = {}


def _build():
    nc = bacc.Bacc("TRN2", target_bir_lowering=False, debug=False)
    f32 = mybir.dt.float32
    f16 = mybir.dt.float16

    x_d = nc.dram_tensor("x", [C, T], f16, kind="ExternalInput")
    wd_d = nc.dram_tensor("wd", [P, WCOLS], f16, kind="ExternalInput")
    w0_d = nc.dram_tensor("w0", [P, RBLK], f32, kind="ExternalInput")
    b_d = nc.dram_tensor("bv", [P, RBLK], f32, kind="ExternalInput")
    y_d = nc.dram_tensor("y", [C, T], f16, kind="ExternalOutput")

    with TileContext(nc) as tc:
        with (
            tc.tile_pool(name="const", bufs=1) as cpool,
            tc.tile_pool(name="io", bufs=IOBUFS) as pool,
            tc.tile_pool(name="ox", bufs=OTBUFS) as opool,
            tc.tile_pool(name="tmp", bufs=3) as tpool,
            tc.tile_pool(name="psum", bufs=4, space="PSUM") as psum_pool,
        ):
            # packed diag weights, two dispatches (block 0 first so its
            # matmuls start ~1us earlier); tap-0 scale + bias ride two tiny
            # f32 tensors
            wt = cpool.tile([P, WCOLS], f16)
            head = NPE * P
            nc.sync.dma_start(out=wt[:, 0:head], in_=wd_d.ap()[:, 0:head])
            w0t = cpool.tile([P, RBLK], f32)
            nc.sync.dma_start(out=w0t, in_=w0_d.ap())
            bt = cpool.tile([P, RBLK], f32)
            nc.sync.dma_start(out=bt, in_=b_d.ap())
            nc.sync.dma_start(out=wt[:, head:], in_=wd_d.ap()[:, head:])

            def wslice(r, k):
                a = (r * NPE + k - 1) * P
                return wt[:, a:a + P]

            # ot-slot store DMAs ride the ACT HWDGE ring (parallel to the SP
            # ring carrying loads). Tile misses the WAR edge "store complete
            # before slot reuse" for ACT-issued DMAs, so add it explicitly:
            # the first write into ot tile m waits on tile (m-OTBUFS)'s last
            # store (ACT-ring DMAs are FIFO, so the last store bounds them).
            tile_last_store = {}
            ot = None
            for r in range(RBLK):
                rows = slice(r * P, (r + 1) * P)
                for i in range(NCHUNK):
                    n = r * NCHUNK + i
                    xt = pool.tile([P, CHUNK + HALO], f16, tag="xt")
                    if i == 0:
                        # memset doesn't support f16; zero via uint16 view
                        nc.vector.memset(xt[:, 0:HALO].bitcast(mybir.dt.uint16), 0)
                        nc.sync.dma_start(out=xt[:, HALO:],
                                          in_=x_d.ap()[rows, 0:CHUNK])
                    else:
                        nc.sync.dma_start(
                            out=xt,
                            in_=x_d.ap()[rows, i * CHUNK - HALO:(i + 1) * CHUNK])

                    pss = []
                    for g in range(2):
                        ps = psum_pool.tile([P, GRAN], f32, tag="ps")
                        pss.append(ps)
                        for sub in range(2):
                            s = g * 2 + sub
                            for k in range(1, K):
                                nc.tensor.matmul(
                                    ps[:, sub * 512:(sub + 1) * 512],
                                    wslice(r, k),
                                    xt[:, s * 512 + k:s * 512 + k + 512],
                                    start=(k == 1), stop=(k == K - 1))
                    tmp = tpool.tile([P, CHUNK], f32, tag="tmp")
                    nc.scalar.activation(
                        tmp, xt[:, 0:CHUNK],
                        mybir.ActivationFunctionType.Identity,
                        bias=bt[:, r:r + 1], scale=w0t[:, r:r + 1])

                    m = n // 2
                    if i % 2 == 0:
                        ot = opool.tile([P, 2 * CHUNK], f16, tag="ot")
                    half = (i % 2) * CHUNK
                    granule_store = n == 0 or n == NTOT - 1
                    for g in range(2):
                        gsl = slice(g * GRAN, (g + 1) * GRAN)
                        osl = slice(half + g * GRAN, half + (g + 1) * GRAN)
                        tt = nc.vector.tensor_add(
                            out=ot[:, osl], in0=tmp[:, gsl], in1=pss[g])
                        if g == 0 and i % 2 == 0 and m >= OTBUFS:
                            add_dep_helper(
                                tt.ins, tile_last_store[m - OTBUFS].ins,
                                reason="ot slot reuse waits for store DMA")
                        if granule_store:
                            # head/tail: store per granule so the store ring
                            # starts early / the tail drains early
                            st = nc.scalar.dma_start(
                                out=y_d.ap()[rows,
                                             i * CHUNK + g * GRAN:
                                             i * CHUNK + (g + 1) * GRAN],
                                in_=ot[:, osl])
                    if granule_store:
                        tile_last_store[m] = st
                    elif n < NSMALL or n == NTOT - 2:
                        # early chunks store per-chunk to ramp the store
                        # ring; chunk 30 stores alone since 31 is per-granule
                        st = nc.scalar.dma_start(
                            out=y_d.ap()[rows, i * CHUNK:(i + 1) * CHUNK],
                            in_=ot[:, half:half + CHUNK])
                        tile_last_store[m] = st
                    elif i % 2 == 1:
                        base = (i - 1) * CHUNK
                        st = nc.scalar.dma_start(
                            out=y_d.ap()[rows, base:base + 2 * CHUNK],
                            in_=ot)
                        tile_last_store[m] = st
    nc.compile()
    return nc


def _host_weights(w, b):
    # wd[p, (r*NPE+k-1)*P + m] = w[r*P+m, 0, k] if p == m else 0 (diag lhsT
    # blocks, taps 1..K-1); tap 0 is applied by the ACT pass via w0.
    wd = np.zeros((P, WCOLS), dtype=np.float16)
    w0 = np.ascontiguousarray(w[:, 0, 0].reshape(RBLK, P).T).astype(np.float32)
    bv = np.ascontiguousarray(b.reshape(RBLK, P).T).astype(np.float32)
    m = np.arange(P)
    for r in range(RBLK):
        for k in range(1, K):
            wd[m, (r * NPE + k - 1) * P + m] = \
                w[r * P + m, 0, k].astype(np.float16)
    return wd, w0, bv


def kernel(x, w, b):
    x = np.asarray(x, dtype=np.float32)
    w = np.asarray(w, dtype=np.float32)
    b = np.asarray(b, dtype=np.float32)

    if "nc" not in _cached:
        _cached["nc"] = _build()
    nc = _cached["nc"]

    wd, w0, bv = _host_weights(w, b)
    x16 = x.astype(np.float16)
    in_maps = [
        {"x": np.ascontiguousarray(x16[j]), "wd": wd, "w0": w0, "bv": bv}
        for j in range(B)
    ]
    res = bass_utils.run_bass_kernel_spmd(nc, in_maps, core_ids=list(range(B)))
    return np.stack([r["y"] for r in res.results], axis=0).astype(np.float32)
